# revision 1
# baseline (speedup 1.0000x reference)
"""Trainium2 Bass kernel for nn_CBModel_46926812676771 (scatter_memory).

Reference semantics: from two pose tensors [32, 18, 2] build four one-hot
heatmap stacks [2, 32, 18, 256, 256]:
  gen_poses[gi]  = heatmap of trunc'd sample-0 coords of pose{gi+1}, replicated over B
  step_poses[si] = heatmap of per-sample interpolated coords p1 + (si+1)*floor((p2-p1)/3)

Sharding: pure data parallel over B (4 samples per core, 8 cores).
Each core writes its 75.5 MB output shard: 288 one-hot [256,256] maps.

Device strategy (memory-roofline bound):
  - per-map scatter target index t = 256*x + y (or -1 if out of bounds) is
    computed on-device from raw (x, y) coords with DVE ops (trunc via the
    1.5*2^23 round trick plus floor/ceil correction).
  - all 288 output rows live one-per-partition in three groups
    (128 + 128 + 32 slots); for each chunk [lo, lo+fj) one DVE tensor_scalar
    computes (iota - t[p]) == -lo, yielding the one-hot values directly.
  - every store is a full-partition HWDGE DMA (8 x 32KB descriptors per
    SDMA engine) so all 16 engines stream evenly.
"""

import numpy as np

H = 256
W = 256
HWSZ = H * W  # 65536
B = 32
C = 18
NCORES = 8
BPC = B // NCORES  # 4
NSTACK = 2  # gen stacks / step stacks
F = 8192  # chunk free-dim size
NCHUNK = HWSZ // F
NROWS = NSTACK * BPC * C  # 144 rows per output tensor per core
TOTROWS = 2 * NROWS  # 288: step rows 0..143, gen rows 144..287
NGROUPS = 3
GROUP_ROWS = (128, 128, 32)
MAGIC = 12582912.0  # 1.5 * 2^23: v + MAGIC lands in [2^23, 2^24) for |v| < 2^22
IOTA0_W = 1024  # width of host-supplied iota prefix

_PROG_CACHE = {}


def _build_program(bufs=4, dual_ring=False, fd=F, fc=F):
    import concourse.bacc as bacc
    import concourse.mybir as mybir
    import concourse.tile as tile

    f32 = mybir.dt.float32
    i32 = mybir.dt.int32
    Op = mybir.AluOpType

    nc = bacc.Bacc(
        "TRN2",
        target_bir_lowering=False,
        debug=False,
        enable_asserts=False,
        num_devices=NCORES,
    )
    coords_d = nc.dram_tensor("coords", [128, 2 * NGROUPS], f32, kind="ExternalInput")
    iota_d = nc.dram_tensor("iota0", [128, IOTA0_W], f32, kind="ExternalInput")
    out_d = nc.dram_tensor("out", [TOTROWS, HWSZ], f32, kind="ExternalOutput")

    out_ap = out_d.ap()
    coords_ap = coords_d.ap()

    with tile.TileContext(nc) as tc:
        with (
            tc.tile_pool(name="const", bufs=1) as const,
            tc.tile_pool(name="outp", bufs=bufs) as outp,
        ):
            coords_sb = const.tile([128, 2 * NGROUPS], f32)
            nc.sync.dma_start(coords_sb[:], coords_ap[:, :])

            # warm iota tile: host supplies 0..IOTA0_W-1, DVE shift-copies
            # double it to WARMW. Separate from the full-width tile so warm
            # chunk compares don't (falsely, tile-granularity deps) wait on
            # the full iota build.
            WARMW = 2048
            iota_w = const.tile([128, WARMW], f32)
            nc.sync.dma_start(iota_w[:, 0:IOTA0_W], iota_d.ap()[:, :])

            # scratch columns, 2*NGROUPS wide each (x and y handled together)
            W6 = 2 * NGROUPS
            sc = const.tile([128, 12 * W6], f32)
            ncol = [0]

            def col():
                c0 = ncol[0]
                ncol[0] += W6
                return sc[:, c0 : c0 + W6]

            v = coords_sb[:, 0:W6]  # x cols 0..2, y cols 3..5
            # trunc toward zero (matches float->int c-cast semantics)
            rn = col()
            nc.vector.tensor_scalar(rn, v, MAGIC, None, Op.add)
            nc.vector.tensor_scalar(rn, rn, -MAGIC, None, Op.add)
            pos = col()
            nc.vector.tensor_scalar(pos, v, 0.0, None, Op.is_ge)
            fcr = col()  # rn > v: round went up; floor needs -1
            nc.vector.tensor_tensor(fcr, rn, v, Op.is_gt)
            cc = col()  # rn < v: round went down; ceil needs +1
            nc.vector.tensor_tensor(cc, rn, v, Op.is_lt)
            m1 = col()
            nc.vector.tensor_tensor(m1, pos, fcr, Op.mult)
            m2 = col()
            nc.vector.tensor_tensor(m2, pos, cc, Op.mult)
            tr = col()
            nc.vector.tensor_tensor(tr, rn, m1, Op.subtract)
            nc.vector.tensor_tensor(tr, tr, cc, Op.add)
            nc.vector.tensor_tensor(tr, tr, m2, Op.subtract)
            cl = col()  # clip to [0, 255]
            nc.vector.tensor_scalar(cl, tr, 0.0, 255.0, Op.max, Op.min)
            vq = col()  # in-bounds per coord: clip is identity
            nc.vector.tensor_tensor(vq, cl, tr, Op.is_equal)
            valid = col()[:, 0:NGROUPS]
            nc.vector.tensor_tensor(
                valid, vq[:, 0:NGROUPS], vq[:, NGROUPS:W6], Op.mult
            )
            # target = valid * (256*xc + yc + 1) - 1   (-1 never matches iota)
            t0 = col()[:, 0:NGROUPS]
            nc.vector.tensor_scalar(
                t0, cl[:, 0:NGROUPS], 256.0, 1.0, Op.mult, Op.add
            )
            nc.vector.tensor_tensor(t0, t0, cl[:, NGROUPS:W6], Op.add)
            nc.vector.tensor_tensor(t0, t0, valid, Op.mult)
            target = col()[:, 0:NGROUPS]
            nc.vector.tensor_scalar(target, t0, -1.0, None, Op.add)

            # grow the warm iota (after prep in DVE order: prep's 3KB coords
            # DMA lands before the 512KB iota prefix)
            n = IOTA0_W
            while n < WARMW:
                nc.vector.tensor_scalar(
                    iota_w[:, n : 2 * n], iota_w[:, 0:n], float(n), None, Op.add
                )
                n *= 2

            ndma = [0]

            def emit_chunk(lo, fj, iota_t, iw):
                hi = lo + fj
                step = min(fc, iw)
                # group 2 (32 rows, half the SDMA engines) first, so the last
                # DMA in flight is always a full-partition one
                for g in (2, 0, 1):
                    rows = GROUP_ROWS[g]
                    r0 = 128 * g
                    ot = outp.tile([128, fd], f32, tag="ot")
                    # one-hot: (iota - target[p]) == -(lo + s)
                    for s in range(0, fj, step):
                        w = min(step, fj - s)
                        nc.vector.tensor_scalar(
                            ot[0:rows, s : s + w],
                            iota_t[0:rows, 0:w],
                            target[0:rows, g : g + 1],
                            float(-(lo + s)),
                            Op.subtract,
                            Op.is_equal,
                        )
                    eng = nc.scalar if (dual_ring and ndma[0] % 2) else nc.sync
                    ndma[0] += 1
                    eng.dma_start(out_ap[r0 : r0 + rows, lo:hi], ot[0:rows, 0:fj])

            # warm-up chunks: sized so each needs only the warm-iota prefix
            # available by then; stores start while the full iota is built
            for lo, fj in [(0, 1024), (1024, 1024), (2048, 2048), (4096, 4096)]:
                emit_chunk(lo, fj, iota_w, WARMW)

            # full-width iota, built from the warm tile
            iota_f = const.tile([128, fc], f32)
            nc.vector.tensor_copy(iota_f[:, 0:WARMW], iota_w[:])
            n = WARMW
            while n < fc:
                nc.vector.tensor_scalar(
                    iota_f[:, n : 2 * n], iota_f[:, 0:n], float(n), None, Op.add
                )
                n *= 2

            off = 8192
            while off % fd:
                emit_chunk(off, 8192, iota_f, fc)
                off += 8192
            while off < HWSZ:
                emit_chunk(off, fd, iota_f, fc)
                off += fd

    nc.compile()
    return nc


def _get_program():
    if "nc" not in _PROG_CACHE:
        _PROG_CACHE["nc"] = _build_program()
    return _PROG_CACHE["nc"]


def _pack_core_inputs(pose1_cor, pose2_cor):
    """Per-core [128, 6] float32 slot coords: cols [x_g0,x_g1,x_g2,y_g0,y_g1,y_g2].

    Output row layout per core (row = 128*g + p):
      rows   0..143: step maps, row = (si*BPC + b)*C + c
      rows 144..287: gen maps,  row = 144 + (gi*BPC + b)*C + c  (same coords for all b)
      rows 288..383: padding (invalid coords -> all-zero, never DMA'd)
    """
    p1 = np.asarray(pose1_cor, np.float32)
    p2 = np.asarray(pose2_cor, np.float32)
    step = np.floor_divide(p2 - p1, np.float32(3.0)).astype(np.float32)
    c1 = p1 + step
    c2 = c1 + step
    # gen maps use sample-0 coords, replicated over b; identical on every core
    gen_unique = np.stack([p1[0], p2[0]], 0)  # [2, C, 2]
    gen_rows = np.broadcast_to(gen_unique[:, None], (NSTACK, BPC, C, 2)).reshape(
        NROWS, 2
    )
    in_maps = []
    for k in range(NCORES):
        sl = slice(k * BPC, (k + 1) * BPC)
        step_rows = np.stack([c1[sl], c2[sl]], 0).reshape(NROWS, 2)  # [144, 2]
        allrows = np.full((NGROUPS * 128, 2), -1.0e9, np.float32)
        allrows[0:NROWS] = step_rows
        allrows[NROWS:TOTROWS] = gen_rows
        g = allrows.reshape(NGROUPS, 128, 2)
        coords = np.empty((128, 2 * NGROUPS), np.float32)
        for gi in range(NGROUPS):
            coords[:, gi] = g[gi, :, 0]
            coords[:, NGROUPS + gi] = g[gi, :, 1]
        in_maps.append({"coords": coords, "iota0": _IOTA0})
    return in_maps


_IOTA0 = np.ascontiguousarray(
    np.broadcast_to(np.arange(IOTA0_W, dtype=np.float32), (128, IOTA0_W))
)


def _assemble(results):
    gen = np.concatenate(
        [r["out"][NROWS:TOTROWS].reshape(NSTACK, BPC, C, H, W) for r in results],
        axis=1,
    )
    step = np.concatenate(
        [r["out"][0:NROWS].reshape(NSTACK, BPC, C, H, W) for r in results], axis=1
    )
    return gen, step


def kernel(pose1_cor, pose2_cor):
    from concourse.bass_utils import run_bass_kernel_spmd

    nc = _get_program()
    in_maps = _pack_core_inputs(pose1_cor, pose2_cor)
    res = run_bass_kernel_spmd(nc, in_maps, core_ids=list(range(NCORES)))
    return _assemble(res.results)

